# revision 22
# baseline (speedup 1.0000x reference)
"""Trainium2 Bass kernel for causal top-8 sparse attention (nn_DGN7).

Math (see reference):
  A    = top-8 strictly-causal neighbours of each row by x.x^T similarity
  attn = softmax over the selected scores, score = (x Wq^T)(x Wk^T)^T/sqrt(32)
  out  = gelu_exact((mix*x + (1-mix)*attn@x) * gain + bias) * (softplus+0.01)

Sharding: 8 cores; core i handles batch i//4 and, for every prefix level
l=1..8, the 128-row tile g = 4*(l-1) + (i%4).  Every core runs an identical
static program over strips of width 512*l (l=1..8); total causal area is
exactly balanced across cores.

Numerics:
  - similarity strip kept in units of 2048*x.x' (selection is scale
    invariant).  Main term (32h).(64h)' in fp16 (exact power-of-2 scalings
    of h=fp16(x)); hi/lo cross terms h.l' + l.h' (l = fp16((x-h)*2048)) via
    ONE fp8e4m3 DoubleRow matmul per 128-chunk (2x PE rate), i.e. sim costs
    16 fp16-equivalent chunk passes instead of 24.  Sim abs error ~1.3 strip
    units (~2e-5 in x.x'/32 units).
  - top-8 via DVE Max8 + match_replace (exact, first-match tie-break).
    Rows whose 8th/9th-candidate gap < 20 strip units are flagged (v8,v9
    exported) and recomputed exactly on the host (~70 of 8192 rows).
  - q/k/score/msg matmuls in fp16; softmax without max-shift (exp biased by
    -4); Z via ACT accum_out; normalisation after the msg matmul.
  - selection mask applied additively to the scores (0 / -3e38).
  - gain/bias/mix folded host-side: xrm = mix*x*gain + bias, gainb=(1-mix)*gain.
Host does layout prep (transposes/fp16/fp8 piece casts), the degenerate t=0
rows, and the flagged near-tie rows.
"""
import math
import numpy as np
import ml_dtypes

import concourse.bass as bass
import concourse.mybir as mybir
from concourse import bacc
from concourse.tile import TileContext
from concourse.bass_utils import run_bass_kernel_spmd

B, T, D = 2, 4096, 1024
DH = 32
P = 128
PANEL = 512
NLEV = 8
NPAN = 8
NCHUNK = D // P          # 8
NCORES = 8
FMIN = float(np.finfo(np.float32).min)
MASKTHR = -1e38
MASKVAL = -3.0e38        # finite in bf16 (FMIN would round to -inf)
SPLIT = 2048.0           # 2^11 lo-piece scale
ESHIFT = -4.0            # exp input bias (fp16 range safety)
GAPTHR = 20.0            # flag threshold, strip units (2048 * x.x')

f32 = mybir.dt.float32
f16 = mybir.dt.float16
bf16 = mybir.dt.bfloat16
f8 = mybir.dt.float8e4
DR = mybir.MatmulPerfMode.DoubleRow
FP8 = ml_dtypes.float8_e4m3

_prog_cache = {}


def _build_program(act_fn=None, use_dr=True):
    if act_fn is None:
        act_fn = mybir.ActivationFunctionType.Gelu
    nc = bacc.Bacc(trn_type="TRN2")

    # ---------------- DRAM I/O ----------------
    d_pan16 = nc.dram_tensor("pan16", [NPAN, P, NCHUNK, PANEL], f16,
                             kind="ExternalInput")      # (64h)^T panels
    d_pan8 = nc.dram_tensor("pan8", [NPAN, P, NCHUNK, 2, PANEL], f8,
                            kind="ExternalInput")       # (l8,h8)^T slabs
    d_xr16 = nc.dram_tensor("xr16", [NLEV, P, NCHUNK, P], f16,
                            kind="ExternalInput")       # (32h)^T own rows
    d_xr8 = nc.dram_tensor("xr8", [NLEV, P, NCHUNK, 2, P], f8,
                           kind="ExternalInput")        # (h8,l8)^T own rows
    d_xbh = nc.dram_tensor("xbh", [NPAN, P, 4, D], f16, kind="ExternalInput")
    d_xrm = nc.dram_tensor("xrm", [NLEV, P, D], f16, kind="ExternalInput")
    d_wq = nc.dram_tensor("wq", [P, NCHUNK, DH], f16, kind="ExternalInput")
    d_wk = nc.dram_tensor("wk", [P, NCHUNK, DH], f16, kind="ExternalInput")
    d_maskdiag = nc.dram_tensor("maskdiag", [P, PANEL], bf16, kind="ExternalInput")
    d_ident16 = nc.dram_tensor("ident16", [P, P], f16, kind="ExternalInput")
    d_identbf = nc.dram_tensor("identbf", [P, P], bf16, kind="ExternalInput")
    d_zerocol = nc.dram_tensor("zerocol", [P, 1], f32, kind="ExternalInput")
    d_scalecol = nc.dram_tensor("scalecol", [P, 1], f32, kind="ExternalInput")
    d_eshift = nc.dram_tensor("eshift", [P, 1], f32, kind="ExternalInput")
    d_out = nc.dram_tensor("out", [NLEV, P, D], f32, kind="ExternalOutput")
    d_v89 = nc.dram_tensor("v89", [NLEV, P, 2], f32, kind="ExternalOutput")

    with TileContext(nc) as tc:
        with tc.tile_pool(name="const", bufs=1) as cpool, \
             tc.tile_pool(name="strips", bufs=1) as spool, \
             tc.tile_pool(name="big", bufs=1) as bpool, \
             tc.tile_pool(name="panels", bufs=2) as ppool, \
             tc.tile_pool(name="attn", bufs=1) as apool, \
             tc.tile_pool(name="msgx", bufs=2) as mpool, \
             tc.tile_pool(name="work", bufs=2) as wpool, \
             tc.tile_pool(name="work1", bufs=1) as w1pool, \
             tc.tile_pool(name="simP", bufs=2, space="PSUM") as simP, \
             tc.tile_pool(name="miscP", bufs=2, space="PSUM") as miscP, \
             tc.tile_pool(name="tranP", bufs=2, space="PSUM") as tranP, \
             tc.tile_pool(name="msgP", bufs=2, space="PSUM") as msgP:

            # ---------------- panel 0 + own-row DMAs first ----------------
            def load_panel(p):
                t16 = ppool.tile([P, NCHUNK, PANEL], f16, tag="pan16")
                t8 = ppool.tile([P, NCHUNK, 2, PANEL], f8, tag="pan8")
                nc.sync.dma_start(t16, d_pan16[p])
                (nc.scalar if p == 0 else nc.sync).dma_start(t8, d_pan8[p])
                return t16, t8

            cur = load_panel(0)

            xr16_sb, xr8_sb = [], []
            for l in range(NLEV):
                t16 = bpool.tile([P, NCHUNK, P], f16, tag=f"xr16_{l}",
                                 name=f"xr16_{l}")
                t8 = bpool.tile([P, NCHUNK, 2, P], f8, tag=f"xr8_{l}",
                                name=f"xr8_{l}")
                nc.gpsimd.dma_start(t16, d_xr16[l])
                nc.scalar.dma_start(t8, d_xr8[l])
                xr16_sb.append(t16)
                xr8_sb.append(t8)

            # ---------------- constants (scalar queue) ----------------
            wq_sb = cpool.tile([P, NCHUNK, DH], f16)
            wk_sb = cpool.tile([P, NCHUNK, DH], f16)
            nc.scalar.dma_start(wq_sb, d_wq.ap())
            nc.scalar.dma_start(wk_sb, d_wk.ap())
            ident16 = cpool.tile([P, P], f16)
            nc.scalar.dma_start(ident16, d_ident16.ap())
            identbf = cpool.tile([P, P], bf16)
            nc.scalar.dma_start(identbf, d_identbf.ap())
            maskdiag = cpool.tile([P, PANEL], bf16)
            nc.scalar.dma_start(maskdiag, d_maskdiag.ap())
            zerocol = cpool.tile([P, 1], f32)
            nc.scalar.dma_start(zerocol, d_zerocol.ap())
            scalecol = cpool.tile([P, 1], f32)
            nc.scalar.dma_start(scalecol, d_scalecol.ap())
            eshiftcol = cpool.tile([P, 1], f32)
            nc.scalar.dma_start(eshiftcol, d_eshift.ap())

            kT_sb = cpool.tile([DH, T], f16)        # k^T, filled per panel
            strip = [spool.tile([P, PANEL * (l + 1)], f32, tag=f"strip{l}",
                                name=f"strip{l}")
                     for l in range(NLEV)]
            qT = [cpool.tile([DH, P], f16, tag=f"qT{l}", name=f"qT{l}")
                  for l in range(NLEV)]
            attnT = apool.tile([P, 4 * NLEV, P], f16, tag="attnT")

            def emit_sim_tile(l, p, p16, p8, critical=False):
                ps = simP.tile([P, PANEL], f32, tag="sim")
                li = l - 1
                last = (p == l - 1)
                n = NCHUNK + NCHUNK + (1 if last else 0)
                i = 0
                for c in range(NCHUNK):
                    nc.tensor.matmul(ps, xr16_sb[li][:, c], p16[:, c],
                                     start=(i == 0), stop=(i == n - 1))
                    i += 1
                if use_dr:
                    for c in range(NCHUNK):
                        nc.tensor.matmul(ps, xr8_sb[li][:, c], p8[:, c],
                                         start=False, stop=(i == n - 1),
                                         perf_mode=DR)
                        i += 1
                else:
                    for c in range(NCHUNK):
                        for s2 in range(2):
                            nc.tensor.matmul(ps, xr8_sb[li][:, c, s2],
                                             p8[:, c, s2],
                                             start=False,
                                             stop=(i == n - 1 and s2 == 1))
                    i += 1
                if last:
                    nc.tensor.matmul(ps, identbf, maskdiag,
                                     start=False, stop=True)
                if critical:
                    # last tile of this level's strip: copy on DVE so the
                    # selection chain isn't stuck behind the ACT queue
                    nc.vector.tensor_copy(
                        strip[li][:, PANEL * p:PANEL * (p + 1)], ps)
                else:
                    nc.scalar.copy(strip[li][:, PANEL * p:PANEL * (p + 1)], ps)

            def issue_selection(l):
                """DVE: top-8 select, flag columns, additive mask (in place)"""
                li = l - 1
                st = strip[li]
                top8 = w1pool.tile([P, 8], f32, tag="top8")
                nc.vector.max(out=top8, in_=st)
                nc.vector.match_replace(out=st, in_to_replace=top8,
                                        in_values=st, imm_value=FMIN)
                v9c = w1pool.tile([P, 1], f32, tag="v9")
                nc.vector.tensor_reduce(out=v9c, in_=st,
                                        op=mybir.AluOpType.max,
                                        axis=mybir.AxisListType.X)
                v8c = w1pool.tile([P, 1], f32, tag="v8")
                nc.vector.tensor_reduce(out=v8c, in_=top8,
                                        op=mybir.AluOpType.min,
                                        axis=mybir.AxisListType.X)
                nc.gpsimd.dma_start(d_v89[li][:, 0:1], v8c)
                nc.gpsimd.dma_start(d_v89[li][:, 1:2], v9c)
                nc.vector.tensor_scalar(st, st, MASKTHR, scalar2=MASKVAL,
                                        op0=mybir.AluOpType.is_gt,
                                        op1=mybir.AluOpType.mult)

            def load_xbh(c):
                xbh = mpool.tile([P, 4, D], f16, tag="xbh")
                (nc.gpsimd if c % 2 else nc.sync).dma_start(xbh, d_xbh[c])
                return xbh

            pending_fin = []   # deferred (li, gh0, gh1, xrm) out-stage tail

            def out_fin():
                """gelu+scale+store for the previous level, emitted after the
                next level's exps so ACT table swaps stay off the PE path"""
                if not pending_fin:
                    return
                li, ghs, xrm = pending_fin.pop()
                for k in range(2):
                    sl = slice(PANEL * k, PANEL * (k + 1))
                    nc.vector.tensor_add(ghs[k], ghs[k], xrm[:, sl])
                    nc.scalar.activation(ghs[k], ghs[k], act_fn,
                                         bias=zerocol, scale=1.0)
                    nc.vector.tensor_scalar_mul(ghs[k], ghs[k], scalecol)
                    nc.gpsimd.dma_start(d_out[li][:, sl], ghs[k])

            def level_compute(l, xbh0):
                """scores/exp/attn^T + msg + out for level l (mask ready)"""
                li = l - 1
                st = strip[li]
                # xrm for the out stage (issued early, gpsimd queue)
                xrm = w1pool.tile([P, D], f16, tag=f"xrm{li % 2}")
                nc.gpsimd.dma_start(xrm, d_xrm[li])
                # --- q^T for this level ---
                qps = miscP.tile([P, PANEL], f32, tag="misc")
                for c in range(NCHUNK):
                    nc.tensor.matmul(qps[:DH, :P], wq_sb[:, c],
                                     xr16_sb[li][:, c],
                                     start=(c == 0), stop=(c == NCHUNK - 1))
                nc.scalar.copy(qT[li], qps[:DH, :P])
                zcols = w1pool.tile([P, NLEV], f32, tag="zcols")
                mp0 = msgP.tile([P, PANEL], f32, tag="msg")
                mp1 = msgP.tile([P, PANEL], f32, tag="msg")
                mps = [mp0, mp1]
                nblk = 4 * l

                xbh_next = xbh0
                for c in range(l):
                    xbh = xbh_next
                    if c + 1 < l:
                        xbh_next = load_xbh(c + 1)
                    sps = miscP.tile([P, PANEL], f32, tag="misc")
                    nc.tensor.matmul(sps, qT[li],
                                     kT_sb[:, PANEL * c:PANEL * (c + 1)],
                                     start=True, stop=(c != l - 1))
                    if c == l - 1:
                        nc.tensor.matmul(sps, identbf, maskdiag,
                                         start=False, stop=True)
                    nc.vector.tensor_add(sps, sps,
                                         st[:, PANEL * c:PANEL * (c + 1)])
                    au = wpool.tile([P, PANEL], f16, tag="au")
                    nc.scalar.activation(au, sps,
                                         mybir.ActivationFunctionType.Exp,
                                         bias=eshiftcol, scale=1.0,
                                         accum_out=zcols[:, c:c + 1])
                    tp = tranP.tile([P, PANEL], f16, tag="tran")
                    for q in range(4):
                        nc.tensor.matmul(tp[:, P * q:P * (q + 1)],
                                         au[:, P * q:P * (q + 1)], ident16,
                                         is_transpose=True,
                                         start=(q == 0), stop=(q == 3))
                    nc.scalar.copy(
                        attnT[:, 4 * c:4 * (c + 1)].rearrange("p b t -> p (b t)"),
                        tp)
                    for sb in range(4):
                        blk = 4 * c + sb
                        for k in range(2):
                            nc.tensor.matmul(
                                mps[k], attnT[:, blk],
                                xbh[:, sb, PANEL * k:PANEL * (k + 1)],
                                start=(blk == 0), stop=(blk == nblk - 1))
                # finish the PREVIOUS level's out stage (ACT gelu lands after
                # this level's exps on the ACT queue)
                out_fin()
                # --- Z -> 1/Z (per-partition column) ---
                zsum = w1pool.tile([P, 1], f32, tag="zsum")
                nc.vector.tensor_reduce(
                    out=zsum, in_=zcols[:, :l], op=mybir.AluOpType.add,
                    axis=mybir.AxisListType.X)
                nc.vector.tensor_scalar_max(zsum, zsum, 1e-30)
                zrec = w1pool.tile([P, 1], f32, tag="zrec")
                nc.vector.reciprocal(zrec, zsum)
                # --- out stage part 1: normalise into SBUF, defer the rest
                # ((1-mix)*gain is pre-folded into xbh host-side) ---
                ghs = []
                for k in range(2):
                    gh = w1pool.tile([P, PANEL], f32, tag=f"g{k}",
                                     name=f"g{k}")
                    nc.vector.tensor_scalar_mul(gh, mps[k], zrec)
                    ghs.append(gh)
                pending_fin.append((li, ghs, xrm))

            # ---------------- main pipeline ----------------
            dexp = w1pool.tile([P, 1], f32, tag="dexp")
            for p in range(NPAN):
                nxt = load_panel(p + 1) if p + 1 < NPAN else None
                xbh0 = load_xbh(0) if p >= 1 else None
                if p >= 1:
                    # dummy: pull the Exp table load off the critical path
                    nc.scalar.activation(dexp, eshiftcol,
                                         mybir.ActivationFunctionType.Exp,
                                         bias=zerocol, scale=1.0)
                p16, p8 = cur
                # level p+1's last tile first, so its strip completes early
                # and its selection overlaps the rest of this iteration
                emit_sim_tile(p + 1, p, p16, p8, critical=True)
                issue_selection(p + 1)
                # k^T panel
                kps = miscP.tile([P, PANEL], f32, tag="misc")
                for c in range(NCHUNK):
                    nc.tensor.matmul(kps[:DH, :], wk_sb[:, c], p16[:, c],
                                     start=(c == 0), stop=(c == NCHUNK - 1))
                nc.scalar.copy(kT_sb[:, PANEL * p:PANEL * (p + 1)], kps[:DH, :])
                for l in range(p + 2, NLEV + 1):
                    emit_sim_tile(l, p, p16, p8)
                if p >= 1:
                    level_compute(p, xbh0)
                cur = nxt
            level_compute(NLEV, load_xbh(0))
            out_fin()

    nc.compile()
    return nc


def _gelu_exact_np(v):
    er = np.array([math.erf(float(t) / math.sqrt(2.0)) for t in v.ravel()],
                  dtype=np.float64).reshape(v.shape)
    return v * 0.5 * (1.0 + er)


def _fix_row(out, xb, W_q, W_k, gain, bias, mix, scale, t):
    """Recompute row t of batch xb exactly (host, fp32 selection/fp64 tail)."""
    kk = min(8, t)
    if kk == 0:
        return  # t=0 handled by caller
    srow = xb[:t] @ xb[t]                       # fp32 similarities (j < t)
    idx = np.argsort(-srow, kind="stable")[:kk]
    q = (xb[t:t + 1] @ W_q.T).astype(np.float64)[0] / math.sqrt(DH)
    kv = (xb[idx] @ W_k.T).astype(np.float64)
    sc = kv @ q
    sc -= sc.max()
    e = np.exp(sc)
    a = e / e.sum()
    msg = a @ xb[idx].astype(np.float64)
    blended = mix * xb[t].astype(np.float64) + (1.0 - mix) * msg
    pre = blended * gain.astype(np.float64) + bias.astype(np.float64)
    out[t] = (_gelu_exact_np(pre) * scale).astype(np.float32)


def kernel(x, W_q, W_k, gain, bias, log_mix, log_scale):
    x = np.ascontiguousarray(np.asarray(x, dtype=np.float32))
    W_q = np.asarray(W_q, dtype=np.float32)
    W_k = np.asarray(W_k, dtype=np.float32)
    gain = np.asarray(gain, dtype=np.float32)
    bias = np.asarray(bias, dtype=np.float32)
    mix = float(1.0 / (1.0 + math.exp(-float(log_mix))))
    scale = float(np.log1p(np.exp(np.float32(log_scale))) + np.float32(0.01))

    if "prog" not in _prog_cache:
        _prog_cache["prog"] = _build_program()
    nc = _prog_cache["prog"]

    # ---- host-side layout prep ----
    xh = x.astype(np.float16)
    hf = xh.astype(np.float32)
    xl = ((x - hf) * SPLIT).astype(np.float16)
    h32 = (hf * 32.0).astype(np.float16)     # exact power-of-2 scalings
    h64 = (hf * 64.0).astype(np.float16)
    h8 = xh.astype(FP8)
    l8 = xl.astype(FP8)

    ident16 = np.eye(P, dtype=np.float16)
    identbf = np.eye(P, dtype=np.float32).astype(ml_dtypes.bfloat16)
    wq = np.ascontiguousarray(
        (W_q / (32.0 * math.sqrt(DH))).T.astype(np.float16)
        .reshape(NCHUNK, P, DH).transpose(1, 0, 2))
    wk = np.ascontiguousarray(
        (W_k / 64.0).T.astype(np.float16)
        .reshape(NCHUNK, P, DH).transpose(1, 0, 2))
    scalecol = np.full((P, 1), scale, dtype=np.float32)
    gainb = ((1.0 - mix) * gain).astype(np.float32)   # folded into xbh

    per_batch = {}
    for b in range(B):
        pan16 = np.ascontiguousarray(
            h64[b].T.reshape(NCHUNK, P, NPAN, PANEL).transpose(2, 1, 0, 3))
        l8T = l8[b].T.reshape(NCHUNK, P, NPAN, PANEL)
        h8T = h8[b].T.reshape(NCHUNK, P, NPAN, PANEL)
        pan8 = np.ascontiguousarray(
            np.stack([l8T, h8T], axis=2).transpose(3, 1, 0, 2, 4))
        xbh = np.ascontiguousarray(
            (x[b] * gainb).astype(np.float16)
            .reshape(NPAN, 4, P, D).transpose(0, 2, 1, 3))
        per_batch[b] = {"pan16": pan16, "pan8": pan8, "xbh": xbh}

    in_maps = []
    for core in range(NCORES):
        b, j = core // 4, core % 4
        rows = np.concatenate(
            [np.arange(P * (4 * l + j), P * (4 * l + j) + P) for l in range(NLEV)])
        xr = x[b][rows].reshape(NLEV, P, D)          # [lev, t, d] fp32
        xr16 = np.ascontiguousarray(
            h32[b][rows].reshape(NLEV, P, NCHUNK, P).transpose(0, 3, 2, 1))
        h8r = h8[b][rows].reshape(NLEV, P, NCHUNK, P)
        l8r = l8[b][rows].reshape(NLEV, P, NCHUNK, P)
        xr8 = np.ascontiguousarray(
            np.stack([h8r, l8r], axis=3).transpose(0, 4, 2, 3, 1))
        xrm = (mix * xr * gain + bias).astype(np.float16)
        md = np.zeros((P, PANEL), dtype=np.float32)
        k_idx = np.arange(P)[:, None]
        s_idx = np.arange(PANEL)[None, :]
        md[s_idx >= k_idx + P * j] = MASKVAL
        in_maps.append({
            **per_batch[b],
            "xr16": xr16, "xr8": xr8, "xrm": xrm,
            "maskdiag": md.astype(ml_dtypes.bfloat16),
            "wq": wq, "wk": wk,
            "ident16": ident16, "identbf": identbf,
            "zerocol": np.zeros((P, 1), dtype=np.float32),
            "scalecol": scalecol,
            "eshift": np.full((P, 1), ESHIFT, dtype=np.float32),
        })

    res = run_bass_kernel_spmd(nc, in_maps, core_ids=list(range(NCORES)))
    _prog_cache["last_results"] = res

    out = np.empty((B, T, D), dtype=np.float32)
    flagged = []
    for core in range(NCORES):
        b, j = core // 4, core % 4
        o = res.results[core]["out"]                 # [lev, t, d]
        v89 = res.results[core]["v89"]               # [lev, t, 2]
        for l in range(NLEV):
            r0 = P * (4 * l + j)
            out[b, r0:r0 + P, :] = o[l]
            gap = v89[l, :, 0].astype(np.float64) - v89[l, :, 1].astype(np.float64)
            for r in np.nonzero(gap < GAPTHR)[0]:
                flagged.append((b, r0 + int(r)))

    # near-tie rows: recompute exactly on host (selection ambiguous on device)
    for b, t in flagged:
        _fix_row(out[b], x[b], W_q, W_k, gain, bias, mix, scale, t)

    # degenerate t=0 rows: uniform attention over ALL positions
    for b in range(B):
        msg0 = x[b].sum(axis=0, dtype=np.float32) * np.float32(1.0 / T)
        blended = np.float32(mix) * x[b, 0] + np.float32(1.0 - mix) * msg0
        pre = blended * gain + bias
        out[b, 0, :] = (_gelu_exact_np(pre.astype(np.float64))
                        * scale).astype(np.float32)
    return out


# revision 27
# speedup vs baseline: 1.0988x; 1.0988x over previous
"""Trainium2 Bass kernel for causal top-8 sparse attention (nn_DGN7).

Math (see reference):
  A    = top-8 strictly-causal neighbours of each row by x.x^T similarity
  attn = softmax over the selected scores, score = (x Wq^T)(x Wk^T)^T/sqrt(32)
  out  = gelu_exact((mix*x + (1-mix)*attn@x) * gain + bias) * (softplus+0.01)

Sharding: 8 cores; core i handles batch i//4 and, for every prefix level
l=1..8, the 128-row tile g = 4*(l-1) + (i%4).  Every core runs an identical
static program over strips of width 512*l (l=1..8); total causal area is
exactly balanced across cores.

Numerics:
  - similarity strip kept in units of 2048*x.x' (selection is scale
    invariant).  Main term (32h).(64h)' in fp16 (exact power-of-2 scalings
    of h=fp16(x)); hi/lo cross terms h.l' + l.h' (l = fp16((x-h)*2048)) via
    ONE fp8e4m3 DoubleRow matmul per 128-chunk (2x PE rate), i.e. sim costs
    16 fp16-equivalent chunk passes instead of 24.  Sim abs error ~1.3 strip
    units (~2e-5 in x.x'/32 units).
  - top-8 via DVE Max8 + match_replace (exact, first-match tie-break).
    Rows whose 8th/9th-candidate gap < 20 strip units are flagged (v8,v9
    exported) and recomputed exactly on the host (~70 of 8192 rows).
  - q/k/score/msg matmuls in fp16; softmax without max-shift (exp biased by
    -4); Z via ACT accum_out; normalisation after the msg matmul.
  - selection mask applied additively to the scores (0 / -3e38).
  - gain/bias/mix folded host-side: xrm = mix*x*gain + bias, gainb=(1-mix)*gain.
Host does layout prep (transposes/fp16/fp8 piece casts), the degenerate t=0
rows, and the flagged near-tie rows.
"""
import math
import numpy as np
import ml_dtypes

import concourse.bass as bass
import concourse.mybir as mybir
from concourse import bacc
from concourse.tile import TileContext
from concourse.bass_utils import run_bass_kernel_spmd

B, T, D = 2, 4096, 1024
DH = 32
P = 128
PANEL = 512
NLEV = 8
NPAN = 8
NCHUNK = D // P          # 8
NCORES = 8
FMIN = float(np.finfo(np.float32).min)
MASKTHR = -1e38
MASKVAL = -3.0e38        # finite in bf16 (FMIN would round to -inf)
SPLIT = 2048.0           # 2^11 lo-piece scale
ESHIFT = -4.0            # exp input bias (fp16 range safety)
GAPTHR = 20.0            # flag threshold, strip units (2048 * x.x')

f32 = mybir.dt.float32
f16 = mybir.dt.float16
bf16 = mybir.dt.bfloat16
f8 = mybir.dt.float8e4
DR = mybir.MatmulPerfMode.DoubleRow
FP8 = ml_dtypes.float8_e4m3

_prog_cache = {}


def _build_program(act_fn=None, use_dr=True):
    if act_fn is None:
        act_fn = mybir.ActivationFunctionType.Gelu
    nc = bacc.Bacc(trn_type="TRN2")

    # ---------------- DRAM I/O ----------------
    d_pan16 = nc.dram_tensor("pan16", [NPAN, P, NCHUNK, PANEL], f16,
                             kind="ExternalInput")      # (64h)^T panels
    d_pan8 = nc.dram_tensor("pan8", [NPAN, P, NCHUNK, 2, PANEL], f8,
                            kind="ExternalInput")       # (l8,h8)^T slabs
    d_xr16 = nc.dram_tensor("xr16", [NLEV, P, NCHUNK, P], f16,
                            kind="ExternalInput")       # (32h)^T own rows
    d_xr8 = nc.dram_tensor("xr8", [NLEV, P, NCHUNK, 2, P], f8,
                           kind="ExternalInput")        # (h8,l8)^T own rows
    d_xbh = nc.dram_tensor("xbh", [NPAN, P, 4, D], f16, kind="ExternalInput")
    d_xrm = nc.dram_tensor("xrm", [NLEV, P, D], f16, kind="ExternalInput")
    d_wq = nc.dram_tensor("wq", [P, NCHUNK, DH], f16, kind="ExternalInput")
    d_wk = nc.dram_tensor("wk", [P, NCHUNK, DH], f16, kind="ExternalInput")
    d_maskdiag = nc.dram_tensor("maskdiag", [P, PANEL], bf16, kind="ExternalInput")
    d_ident16 = nc.dram_tensor("ident16", [P, P], f16, kind="ExternalInput")
    d_identbf = nc.dram_tensor("identbf", [P, P], bf16, kind="ExternalInput")
    d_zerocol = nc.dram_tensor("zerocol", [P, 1], f32, kind="ExternalInput")
    d_scalecol = nc.dram_tensor("scalecol", [P, 1], f32, kind="ExternalInput")
    d_eshift = nc.dram_tensor("eshift", [P, 1], f32, kind="ExternalInput")
    d_out = nc.dram_tensor("out", [NLEV, P, D], f32, kind="ExternalOutput")
    d_v89 = nc.dram_tensor("v89", [NLEV, P, 2], f32, kind="ExternalOutput")

    with TileContext(nc) as tc:
        with tc.tile_pool(name="const", bufs=1) as cpool, \
             tc.tile_pool(name="strips", bufs=1) as spool, \
             tc.tile_pool(name="big", bufs=1) as bpool, \
             tc.tile_pool(name="panels", bufs=2) as ppool, \
             tc.tile_pool(name="attn", bufs=1) as apool, \
             tc.tile_pool(name="msgx", bufs=2) as mpool, \
             tc.tile_pool(name="work", bufs=2) as wpool, \
             tc.tile_pool(name="work1", bufs=1) as w1pool, \
             tc.tile_pool(name="simP", bufs=2, space="PSUM") as simP, \
             tc.tile_pool(name="miscP", bufs=2, space="PSUM") as miscP, \
             tc.tile_pool(name="tranP", bufs=2, space="PSUM") as tranP, \
             tc.tile_pool(name="msgP", bufs=2, space="PSUM") as msgP:

            # ---------------- panel 0 + own-row DMAs first ----------------
            def load_panel(p):
                t16 = ppool.tile([P, NCHUNK, PANEL], f16, tag="pan16")
                t8 = ppool.tile([P, NCHUNK, 2, PANEL], f8, tag="pan8")
                nc.sync.dma_start(t16, d_pan16[p])
                (nc.scalar if p == 0 else nc.sync).dma_start(t8, d_pan8[p])
                return t16, t8

            # ---------------- constants first (tiny, feed first sim) -------
            wq_sb = cpool.tile([P, NCHUNK, DH], f16)
            wk_sb = cpool.tile([P, NCHUNK, DH], f16)
            nc.scalar.dma_start(wq_sb, d_wq.ap())
            nc.scalar.dma_start(wk_sb, d_wk.ap())
            ident16 = cpool.tile([P, P], f16)
            nc.scalar.dma_start(ident16, d_ident16.ap())
            identbf = cpool.tile([P, P], bf16)
            nc.scalar.dma_start(identbf, d_identbf.ap())
            maskdiag = cpool.tile([P, PANEL], bf16)
            nc.scalar.dma_start(maskdiag, d_maskdiag.ap())
            zerocol = cpool.tile([P, 1], f32)
            nc.scalar.dma_start(zerocol, d_zerocol.ap())
            scalecol = cpool.tile([P, 1], f32)
            nc.scalar.dma_start(scalecol, d_scalecol.ap())
            eshiftcol = cpool.tile([P, 1], f32)
            nc.scalar.dma_start(eshiftcol, d_eshift.ap())

            cur = load_panel(0)

            xr16_sb, xr8_sb = [], []
            for l in range(NLEV):
                t16 = bpool.tile([P, NCHUNK, P], f16, tag=f"xr16_{l}",
                                 name=f"xr16_{l}")
                t8 = bpool.tile([P, NCHUNK, 2, P], f8, tag=f"xr8_{l}",
                                name=f"xr8_{l}")
                nc.gpsimd.dma_start(t16, d_xr16[l])
                nc.scalar.dma_start(t8, d_xr8[l])
                xr16_sb.append(t16)
                xr8_sb.append(t8)

            kT_sb = cpool.tile([DH, T], f16)        # k^T, filled per panel
            strip = [spool.tile([P, PANEL * (l + 1)], f32, tag=f"strip{l}",
                                name=f"strip{l}")
                     for l in range(NLEV)]
            qT = [cpool.tile([DH, P], f16, tag=f"qT{l}", name=f"qT{l}")
                  for l in range(NLEV)]
            attnT = apool.tile([P, 4 * NLEV, P], f16, tag="attnT")

            def emit_sim_tile(l, p, p16, p8, critical=False):
                ps = simP.tile([P, PANEL], f32, tag="sim")
                li = l - 1
                last = (p == l - 1)
                n = NCHUNK + NCHUNK + (1 if last else 0)
                i = 0
                for c in range(NCHUNK):
                    nc.tensor.matmul(ps, xr16_sb[li][:, c], p16[:, c],
                                     start=(i == 0), stop=(i == n - 1))
                    i += 1
                if use_dr:
                    for c in range(NCHUNK):
                        nc.tensor.matmul(ps, xr8_sb[li][:, c], p8[:, c],
                                         start=False, stop=(i == n - 1),
                                         perf_mode=DR)
                        i += 1
                else:
                    for c in range(NCHUNK):
                        for s2 in range(2):
                            nc.tensor.matmul(ps, xr8_sb[li][:, c, s2],
                                             p8[:, c, s2],
                                             start=False,
                                             stop=(i == n - 1 and s2 == 1))
                    i += 1
                if last:
                    nc.tensor.matmul(ps, identbf, maskdiag,
                                     start=False, stop=True)
                if critical:
                    # last tile of this level's strip: copy on DVE so the
                    # selection chain isn't stuck behind the ACT queue
                    nc.vector.tensor_copy(
                        strip[li][:, PANEL * p:PANEL * (p + 1)], ps)
                else:
                    nc.scalar.copy(strip[li][:, PANEL * p:PANEL * (p + 1)], ps)

            def issue_selection(l):
                """DVE: top-8 select, flag columns, additive mask (in place)"""
                li = l - 1
                st = strip[li]
                top8 = w1pool.tile([P, 8], f32, tag="top8")
                nc.vector.max(out=top8, in_=st)
                nc.vector.match_replace(out=st, in_to_replace=top8,
                                        in_values=st, imm_value=FMIN)
                v9c = w1pool.tile([P, 1], f32, tag="v9")
                nc.vector.tensor_reduce(out=v9c, in_=st,
                                        op=mybir.AluOpType.max,
                                        axis=mybir.AxisListType.X)
                v8c = w1pool.tile([P, 1], f32, tag="v8")
                nc.vector.tensor_reduce(out=v8c, in_=top8,
                                        op=mybir.AluOpType.min,
                                        axis=mybir.AxisListType.X)
                nc.gpsimd.dma_start(d_v89[li][:, 0:1], v8c)
                nc.gpsimd.dma_start(d_v89[li][:, 1:2], v9c)
                nc.vector.tensor_scalar(st, st, MASKTHR, scalar2=MASKVAL,
                                        op0=mybir.AluOpType.is_gt,
                                        op1=mybir.AluOpType.mult)

            # msg panels 0/1 are read by 8/7 levels: keep them resident
            xres = [cpool.tile([P, 4, D], f16, tag=f"xres{c}",
                               name=f"xres{c}") for c in range(2)]

            def load_xbh(c):
                xbh = mpool.tile([P, 4, D], f16, tag="xbh")
                (nc.gpsimd if c % 2 else nc.sync).dma_start(xbh, d_xbh[c])
                return xbh

            pending_fin = []   # deferred (li, gh0, gh1, xrm) out-stage tail

            def out_fin():
                """gelu+scale+store for the previous level, emitted after the
                next level's exps so ACT table swaps stay off the PE path"""
                if not pending_fin:
                    return
                li, ghs, xrm = pending_fin.pop()
                for k in range(2):
                    sl = slice(PANEL * k, PANEL * (k + 1))
                    nc.vector.tensor_add(ghs[k], ghs[k], xrm[:, sl])
                    nc.scalar.activation(ghs[k], ghs[k], act_fn,
                                         bias=zerocol, scale=1.0)
                    nc.vector.tensor_scalar_mul(ghs[k], ghs[k], scalecol)
                    nc.gpsimd.dma_start(d_out[li][:, sl], ghs[k])

            def level_compute(l, xbh0):
                """scores/exp/attn^T + msg + out for level l (mask ready)"""
                li = l - 1
                st = strip[li]
                # xrm for the out stage (issued early, gpsimd queue)
                xrm = w1pool.tile([P, D], f16, tag=f"xrm{li % 2}")
                nc.gpsimd.dma_start(xrm, d_xrm[li])
                # --- q^T for this level ---
                qps = miscP.tile([P, PANEL], f32, tag="misc")
                for c in range(NCHUNK):
                    nc.tensor.matmul(qps[:DH, :P], wq_sb[:, c],
                                     xr16_sb[li][:, c],
                                     start=(c == 0), stop=(c == NCHUNK - 1))
                nc.scalar.copy(qT[li], qps[:DH, :P])
                zcols = w1pool.tile([P, NLEV], f32, tag="zcols")
                mp0 = msgP.tile([P, PANEL], f32, tag="msg")
                mp1 = msgP.tile([P, PANEL], f32, tag="msg")
                mps = [mp0, mp1]
                nblk = 4 * l

                xbh_next = xbh0
                for c in range(l):
                    if c < 2:
                        xbh = xres[c]
                    else:
                        xbh = xbh_next
                        if c + 1 < l:
                            xbh_next = load_xbh(c + 1)
                    sps = miscP.tile([P, PANEL], f32, tag="misc")
                    nc.tensor.matmul(sps, qT[li],
                                     kT_sb[:, PANEL * c:PANEL * (c + 1)],
                                     start=True, stop=(c != l - 1))
                    if c == l - 1:
                        nc.tensor.matmul(sps, identbf, maskdiag,
                                         start=False, stop=True)
                    nc.vector.tensor_add(sps, sps,
                                         st[:, PANEL * c:PANEL * (c + 1)])
                    au = wpool.tile([P, PANEL], f16, tag="au")
                    nc.scalar.activation(au, sps,
                                         mybir.ActivationFunctionType.Exp,
                                         bias=eshiftcol, scale=1.0,
                                         accum_out=zcols[:, c:c + 1])
                    tp = tranP.tile([P, PANEL], f16, tag="tran")
                    for q in range(4):
                        nc.tensor.matmul(tp[:, P * q:P * (q + 1)],
                                         au[:, P * q:P * (q + 1)], ident16,
                                         is_transpose=True,
                                         start=(q == 0), stop=(q == 3))
                    nc.scalar.copy(
                        attnT[:, 4 * c:4 * (c + 1)].rearrange("p b t -> p (b t)"),
                        tp)
                    for sb in range(4):
                        blk = 4 * c + sb
                        for k in range(2):
                            nc.tensor.matmul(
                                mps[k], attnT[:, blk],
                                xbh[:, sb, PANEL * k:PANEL * (k + 1)],
                                start=(blk == 0), stop=(blk == nblk - 1))
                # finish the PREVIOUS level's out stage (ACT gelu lands after
                # this level's exps on the ACT queue)
                out_fin()
                # --- Z -> 1/Z (per-partition column) ---
                zsum = w1pool.tile([P, 1], f32, tag="zsum")
                nc.vector.tensor_reduce(
                    out=zsum, in_=zcols[:, :l], op=mybir.AluOpType.add,
                    axis=mybir.AxisListType.X)
                nc.vector.tensor_scalar_max(zsum, zsum, 1e-30)
                zrec = w1pool.tile([P, 1], f32, tag="zrec")
                nc.vector.reciprocal(zrec, zsum)
                # --- out stage part 1: normalise into SBUF, defer the rest
                # ((1-mix)*gain is pre-folded into xbh host-side) ---
                ghs = []
                for k in range(2):
                    gh = w1pool.tile([P, PANEL], f32, tag=f"g{k}",
                                     name=f"g{k}")
                    nc.vector.tensor_scalar_mul(gh, mps[k], zrec)
                    ghs.append(gh)
                pending_fin.append((li, ghs, xrm))

            # ---------------- main pipeline ----------------
            dexp = w1pool.tile([P, 1], f32, tag="dexp")
            for p in range(NPAN):
                nxt = load_panel(p + 1) if p + 1 < NPAN else None
                xbh0 = load_xbh(2) if p >= 3 else None
                if p >= 1:
                    # dummy: pull the Exp table load off the critical path
                    nc.scalar.activation(dexp, eshiftcol,
                                         mybir.ActivationFunctionType.Exp,
                                         bias=zerocol, scale=1.0)
                p16, p8 = cur
                # level p+1's last tile first, so its strip completes early
                # and its selection overlaps the rest of this iteration
                emit_sim_tile(p + 1, p, p16, p8, critical=True)
                issue_selection(p + 1)
                # k^T panel
                kps = miscP.tile([P, PANEL], f32, tag="misc")
                for c in range(NCHUNK):
                    nc.tensor.matmul(kps[:DH, :], wk_sb[:, c], p16[:, c],
                                     start=(c == 0), stop=(c == NCHUNK - 1))
                nc.scalar.copy(kT_sb[:, PANEL * p:PANEL * (p + 1)], kps[:DH, :])
                for l in range(p + 2, NLEV + 1):
                    emit_sim_tile(l, p, p16, p8)
                if p == 0:
                    # resident msg panels, loaded once during iter-0 compute
                    nc.sync.dma_start(xres[0], d_xbh[0])
                    nc.gpsimd.dma_start(xres[1], d_xbh[1])
                if p >= 1:
                    level_compute(p, xbh0)
                cur = nxt
            level_compute(NLEV, load_xbh(2))
            out_fin()

    nc.compile()
    return nc


def _gelu_exact_np(v):
    er = np.array([math.erf(float(t) / math.sqrt(2.0)) for t in v.ravel()],
                  dtype=np.float64).reshape(v.shape)
    return v * 0.5 * (1.0 + er)


def _fix_row(out, xb, W_q, W_k, gain, bias, mix, scale, t):
    """Recompute row t of batch xb exactly (host, fp32 selection/fp64 tail)."""
    kk = min(8, t)
    if kk == 0:
        return  # t=0 handled by caller
    srow = xb[:t] @ xb[t]                       # fp32 similarities (j < t)
    idx = np.argsort(-srow, kind="stable")[:kk]
    q = (xb[t:t + 1] @ W_q.T).astype(np.float64)[0] / math.sqrt(DH)
    kv = (xb[idx] @ W_k.T).astype(np.float64)
    sc = kv @ q
    sc -= sc.max()
    e = np.exp(sc)
    a = e / e.sum()
    msg = a @ xb[idx].astype(np.float64)
    blended = mix * xb[t].astype(np.float64) + (1.0 - mix) * msg
    pre = blended * gain.astype(np.float64) + bias.astype(np.float64)
    out[t] = (_gelu_exact_np(pre) * scale).astype(np.float32)


def kernel(x, W_q, W_k, gain, bias, log_mix, log_scale):
    x = np.ascontiguousarray(np.asarray(x, dtype=np.float32))
    W_q = np.asarray(W_q, dtype=np.float32)
    W_k = np.asarray(W_k, dtype=np.float32)
    gain = np.asarray(gain, dtype=np.float32)
    bias = np.asarray(bias, dtype=np.float32)
    mix = float(1.0 / (1.0 + math.exp(-float(log_mix))))
    scale = float(np.log1p(np.exp(np.float32(log_scale))) + np.float32(0.01))

    if "prog" not in _prog_cache:
        _prog_cache["prog"] = _build_program()
    nc = _prog_cache["prog"]

    # ---- host-side layout prep ----
    xh = x.astype(np.float16)
    hf = xh.astype(np.float32)
    xl = ((x - hf) * SPLIT).astype(np.float16)
    h32 = (hf * 32.0).astype(np.float16)     # exact power-of-2 scalings
    h64 = (hf * 64.0).astype(np.float16)
    h8 = xh.astype(FP8)
    l8 = xl.astype(FP8)

    ident16 = np.eye(P, dtype=np.float16)
    identbf = np.eye(P, dtype=np.float32).astype(ml_dtypes.bfloat16)
    wq = np.ascontiguousarray(
        (W_q / (32.0 * math.sqrt(DH))).T.astype(np.float16)
        .reshape(NCHUNK, P, DH).transpose(1, 0, 2))
    wk = np.ascontiguousarray(
        (W_k / 64.0).T.astype(np.float16)
        .reshape(NCHUNK, P, DH).transpose(1, 0, 2))
    scalecol = np.full((P, 1), scale, dtype=np.float32)
    gainb = ((1.0 - mix) * gain).astype(np.float32)   # folded into xbh

    per_batch = {}
    for b in range(B):
        pan16 = np.ascontiguousarray(
            h64[b].T.reshape(NCHUNK, P, NPAN, PANEL).transpose(2, 1, 0, 3))
        l8T = l8[b].T.reshape(NCHUNK, P, NPAN, PANEL)
        h8T = h8[b].T.reshape(NCHUNK, P, NPAN, PANEL)
        pan8 = np.ascontiguousarray(
            np.stack([l8T, h8T], axis=2).transpose(3, 1, 0, 2, 4))
        xbh = np.ascontiguousarray(
            (x[b] * gainb).astype(np.float16)
            .reshape(NPAN, 4, P, D).transpose(0, 2, 1, 3))
        per_batch[b] = {"pan16": pan16, "pan8": pan8, "xbh": xbh}

    in_maps = []
    for core in range(NCORES):
        b, j = core // 4, core % 4
        rows = np.concatenate(
            [np.arange(P * (4 * l + j), P * (4 * l + j) + P) for l in range(NLEV)])
        xr = x[b][rows].reshape(NLEV, P, D)          # [lev, t, d] fp32
        xr16 = np.ascontiguousarray(
            h32[b][rows].reshape(NLEV, P, NCHUNK, P).transpose(0, 3, 2, 1))
        h8r = h8[b][rows].reshape(NLEV, P, NCHUNK, P)
        l8r = l8[b][rows].reshape(NLEV, P, NCHUNK, P)
        xr8 = np.ascontiguousarray(
            np.stack([h8r, l8r], axis=3).transpose(0, 4, 2, 3, 1))
        xrm = (mix * xr * gain + bias).astype(np.float16)
        md = np.zeros((P, PANEL), dtype=np.float32)
        k_idx = np.arange(P)[:, None]
        s_idx = np.arange(PANEL)[None, :]
        md[s_idx >= k_idx + P * j] = MASKVAL
        in_maps.append({
            **per_batch[b],
            "xr16": xr16, "xr8": xr8, "xrm": xrm,
            "maskdiag": md.astype(ml_dtypes.bfloat16),
            "wq": wq, "wk": wk,
            "ident16": ident16, "identbf": identbf,
            "zerocol": np.zeros((P, 1), dtype=np.float32),
            "scalecol": scalecol,
            "eshift": np.full((P, 1), ESHIFT, dtype=np.float32),
        })

    res = run_bass_kernel_spmd(nc, in_maps, core_ids=list(range(NCORES)))
    _prog_cache["last_results"] = res

    out = np.empty((B, T, D), dtype=np.float32)
    flagged = []
    for core in range(NCORES):
        b, j = core // 4, core % 4
        o = res.results[core]["out"]                 # [lev, t, d]
        v89 = res.results[core]["v89"]               # [lev, t, 2]
        for l in range(NLEV):
            r0 = P * (4 * l + j)
            out[b, r0:r0 + P, :] = o[l]
            gap = v89[l, :, 0].astype(np.float64) - v89[l, :, 1].astype(np.float64)
            for r in np.nonzero(gap < GAPTHR)[0]:
                flagged.append((b, r0 + int(r)))

    # near-tie rows: recompute exactly on host (selection ambiguous on device)
    for b, t in flagged:
        _fix_row(out[b], x[b], W_q, W_k, gain, bias, mix, scale, t)

    # degenerate t=0 rows: uniform attention over ALL positions
    for b in range(B):
        msg0 = x[b].sum(axis=0, dtype=np.float32) * np.float32(1.0 / T)
        blended = np.float32(mix) * x[b, 0] + np.float32(1.0 - mix) * msg0
        pre = blended * gain + bias
        out[b, 0, :] = (_gelu_exact_np(pre.astype(np.float64))
                        * scale).astype(np.float32)
    return out
